# revision 43
# baseline (speedup 1.0000x reference)
"""ActorCritic segment-reduce kernel for 8 TRN2 NeuronCores.

Strategy (data-parallel over graph batch B=512 -> 64 graphs/core):
  - The critic MLP is evaluated ONLY on gathered rows (64 selected-node
    rows + 2048 next-node rows per core), not on all 102400 node rows -
    that sparsity is the headroom of this problem.
  - Embeddings and weights are cast to bf16 (matmul compute precision with
    f32 PSUM accumulation; ~3e-3 rel err vs the 2e-2 gate). The segment
    gathers are per-128-row indirect DMAs on the gpsimd SWDGE; gathered
    rows are PE-transposed so the contract dim lands on partitions.
  - Critic layer 2 (relu(H) @ Wc2) is one fused DVE scalar_tensor_tensor
    per row tile: out=(max(psum,0))*Wc2_rep with accum_out emitting the
    row dot product straight from PSUM - no separate relu/mul/reduce.
  - Segment max over K=32 next nodes: the host orders gather rows k-major
    (row = k*64 + b) so the seg-max becomes one free-dim reduce over 16
    stacked result columns plus a single cross-partition-half max.
  - Actor softmax uses a fixed shift exp(ml - 16) so the exponentials fuse
    into the same pass as each 512-column logit chunk; the scalar-engine
    Exp op emits the row sum via accum_out. logp[b, xfers[b]] is a 4-byte
    indirect gather from a DRAM round-trip of the masked logits. The final
    log()/divide on [B]-sized reductions happens on the host during
    unsharding (together with the entropy mean over B).
  - Actor column chunks are interleaved between critic row tiles so the PE
    instruction stream stays dense; ~24 junk matmuls at kernel start plus
    small junk bursts in the early ramp keep the PE HAM clock at 2.4 GHz.
  - All loads are consolidated into one DMA per tensor, split across the
    sync and scalar HWDGE rings, ordered so the gather indices and
    identity land first; rank-1 (ones x bias) bias matmuls are compiled
    only when the bias vector is nonzero (they are zero in practice).
  - No cross-core communication at all: per-core [64, 5] outputs
    (values, next_values, raw xfer logit, sum-exp, sum e*ml) are
    concatenated and combined on the host.
"""
import numpy as np

import concourse.bass as bass
import concourse.mybir as mybir
import concourse.tile as tile
from concourse import bacc
from concourse.bass import IndirectOffsetOnAxis
from concourse.bass_utils import run_bass_kernel_spmd

F32 = mybir.dt.float32
BF16 = mybir.dt.bfloat16
I32 = mybir.dt.int32
U8 = mybir.dt.uint8
AF = mybir.ActivationFunctionType
OP = mybir.AluOpType

B, N, D = 512, 200, 256
HC, HA, A = 512, 512, 4096
K = 32
NCORES = 8
BL = B // NCORES            # 64 graphs per core
RN = BL * K                 # 2048 gathered next rows per core
NT = RN // 128              # 16 row tiles of gathered next rows
P = 128
ACH = A // 512              # 8 actor column chunks of 512
M0 = 16.0                   # fixed log-sum-exp shift (logits are O(5))

_COMPILED = {}


def _build(use_bc1, use_ba2):
    nc = bacc.Bacc("TRN2", target_bir_lowering=False, debug=False,
                   num_devices=NCORES)

    ge = nc.dram_tensor("ge", [BL * N, D], BF16, kind="ExternalInput")
    nge = nc.dram_tensor("nge", [BL * N, D], BF16, kind="ExternalInput")
    wc1 = nc.dram_tensor("wc1", [D, HC], BF16, kind="ExternalInput")
    wa1 = nc.dram_tensor("wa1", [D, HA], BF16, kind="ExternalInput")
    wa2 = nc.dram_tensor("wa2", [HA, A], BF16, kind="ExternalInput")
    masks = nc.dram_tensor("masks", [BL, A], U8, kind="ExternalInput")
    # aux32: col0 xf_idx (i32), cols1-4 ba1 partition-major, col5 term,
    # col6 sel_idx (i32), cols7-22 next-tile idx (i32)
    aux32 = nc.dram_tensor("aux32", [P, 23], F32, kind="ExternalInput")
    # rowc: wc2(512) | bc2(1)
    rowc = nc.dram_tensor("rowc", [513], F32, kind="ExternalInput")
    # biasb: bc1(512) | ba2(4096) in bf16
    biasb = nc.dram_tensor("biasb", [4608], BF16, kind="ExternalInput")
    ident_d = nc.dram_tensor("ident_d", [P, P], BF16, kind="ExternalInput")

    out5 = nc.dram_tensor("out5", [BL, 5], F32, kind="ExternalOutput")

    ml_dram = nc.dram_tensor("ml_dram", [BL * A, 1], F32)  # internal

    with tile.TileContext(nc) as tc:
        with (
            tc.tile_pool(name="const", bufs=1) as const,
            tc.tile_pool(name="work", bufs=6) as work,
            tc.tile_pool(name="junk", bufs=4) as junkp,
            tc.tile_pool(name="small", bufs=8) as small,
            tc.tile_pool(name="pm", bufs=5, space="PSUM") as pm,
            tc.tile_pool(name="pt", bufs=2, space="PSUM") as pt,
            tc.tile_pool(name="pa", bufs=1, space="PSUM") as pa,
        ):
            # ---- PE warm-up + ACT table preload ----
            wdum = const.tile([P, 512], BF16)
            nc.vector.memset(wdum[:], 0.0)
            pdum = pm.tile([P, 512], F32, tag="mm")
            for _ in range(24):
                nc.tensor.matmul(out=pdum[:], lhsT=wdum[:, 0:P], rhs=wdum[:],
                                 start=True, stop=True)
            dl = small.tile([1, 2], F32)
            nc.vector.memset(dl[:], 2.0)
            nc.scalar.activation(out=dl[:, 0:1], in_=dl[:, 1:2], func=AF.Exp)

            # ---- consolidated loads ----
            aux_t = const.tile([P, 23], F32)
            nc.sync.dma_start(out=aux_t[:], in_=aux32[:, :])
            ident = const.tile([P, P], BF16)
            nc.sync.dma_start(out=ident[:], in_=ident_d[:, :])
            wc2_rep = const.tile([P, HC], F32)
            nc.sync.dma_start(out=wc2_rep[:], in_=bass.AP(
                tensor=rowc, offset=0, ap=[[0, P], [1, HC]]))
            bc2_rep = const.tile([P, 1], F32)
            nc.sync.dma_start(out=bc2_rep[:], in_=bass.AP(
                tensor=rowc, offset=HC, ap=[[0, P], [1, 1]]))
            wc1_t = const.tile([P, 2, HC], BF16)
            nc.sync.dma_start(out=wc1_t[:],
                              in_=wc1.ap().rearrange("(c p) h -> p c h", p=P))
            masks_sb = const.tile([BL, A], U8)
            nc.sync.dma_start(out=masks_sb[:], in_=masks[:, :])
            bias_t = const.tile([1, 4608], BF16)
            nc.sync.dma_start(out=bias_t[:], in_=biasb[None, :])
            wa2_t = const.tile([P, 4, A], BF16)
            wa2_r = wa2.ap().rearrange("(c p) a -> p c a", p=P)
            nc.sync.dma_start(out=wa2_t[:, :, 2048:3072], in_=wa2_r[:, :, 2048:3072])
            nc.sync.dma_start(out=wa2_t[:, :, 3072:4096], in_=wa2_r[:, :, 3072:4096])
            wa1_t = const.tile([P, 2, HA], BF16)
            nc.scalar.dma_start(out=wa1_t[:],
                                in_=wa1.ap().rearrange("(c p) h -> p c h", p=P))
            nc.scalar.dma_start(out=wa2_t[:, :, 0:1024], in_=wa2_r[:, :, 0:1024])
            nc.scalar.dma_start(out=wa2_t[:, :, 1024:2048], in_=wa2_r[:, :, 1024:2048])

            bc1_r = bias_t[:, 0:HC]
            ba2_r = bias_t[:, HC:HC + A]
            xf_i = aux_t[:BL, 0:1].bitcast(I32)
            sel_i = aux_t[:BL, 6:7].bitcast(I32)
            nidx_i = [aux_t[:, 7 + t:8 + t].bitcast(I32) for t in range(NT)]
            ones = const.tile([1, P], BF16)
            nc.vector.memset(ones[:], 1.0)
            nm0 = const.tile([P, 1], F32)
            nc.vector.memset(nm0[:], -M0)


            # ---- gathers: plain indirect DMA (no ucode library), one call
            # per 128 rows; bf16 rows then PE-transposed per tile ----
            xsel = const.tile([BL, D], BF16)
            nc.gpsimd.indirect_dma_start(
                out=xsel[:], out_offset=None, in_=ge[:, :],
                in_offset=IndirectOffsetOnAxis(ap=sel_i, axis=0))
            xrows = []
            for t6 in range(NT):
                xrt = const.tile([P, D], BF16, tag=f"xr{t6}")
                xrows.append(xrt)
                nc.gpsimd.indirect_dma_start(
                    out=xrt[:], out_offset=None, in_=nge[:, :],
                    in_offset=IndirectOffsetOnAxis(ap=nidx_i[t6], axis=0))

            def junk_mm(n=2):
                pj = pm.tile([P, 512], F32, tag="mm")
                for _ in range(n):
                    nc.tensor.matmul(out=pj[:], lhsT=wdum[:, 0:P], rhs=wdum[:],
                                     start=True, stop=True)

            selT_t = const.tile([P, 2, BL], BF16)
            tsel = pt.tile([P, 2, BL], BF16, tag="tp")
            for c in range(2):
                nc.tensor.transpose(out=tsel[:, c, :],
                                    in_=xsel[:, c * P:(c + 1) * P],
                                    identity=ident[:BL, :BL])
            nc.vector.tensor_copy(out=selT_t[:], in_=tsel[:])
            selT = selT_t[:, :, :]

            # =============== critic on sel rows (values) ========
            out5_t = small.tile([BL, 5], F32)
            ps = pm.tile([BL, HC], F32, tag="mm")
            nc.tensor.matmul(out=ps[:], lhsT=selT[:, 0, :], rhs=wc1_t[:, 0, :],
                             start=True, stop=False)
            nc.tensor.matmul(out=ps[:], lhsT=selT[:, 1, :], rhs=wc1_t[:, 1, :],
                             start=False, stop=not use_bc1)
            if use_bc1:
                nc.tensor.matmul(out=ps[:], lhsT=ones[:, :BL], rhs=bc1_r,
                                 start=False, stop=True)
            jt = junkp.tile([P, HC], F32, tag="junk")
            vsel = small.tile([BL, 1], F32)
            nc.vector.scalar_tensor_tensor(
                out=jt[:BL, :], in0=ps[:], scalar=0.0, in1=wc2_rep[:BL, :],
                op0=OP.max, op1=OP.mult, accum_out=vsel[:])
            nc.vector.tensor_scalar_add(out5_t[:, 0:1], vsel[:], bc2_rep[:BL, :1])

            # ====== critic next tiles interleaved with actor chunks ======
            v_all = small.tile([P, NT], F32)
            ml_all = const.tile([BL, A], F32)
            s_all = small.tile([BL, ACH], F32)
            u_all = small.tile([BL, ACH], F32)
            ml_view = ml_dram.ap().rearrange("(b a) one -> b (a one)", b=BL)
            ha = const.tile([P, 4, BL], BF16)  # H_a laid out [h, b]

            def critic_tile(t):
                xw = work.tile([P, 2, P], BF16, tag="xt")
                tpn = pt.tile([P, 2, P], BF16, tag="tp")
                for c in range(2):
                    nc.tensor.transpose(
                        out=tpn[:, c, :],
                        in_=xrows[t][:, c * P:(c + 1) * P],
                        identity=ident[:])
                nc.vector.tensor_copy(out=xw[:], in_=tpn[:])
                xT = xw
                pn = pm.tile([P, HC], F32, tag="mm")
                nc.tensor.matmul(out=pn[:], lhsT=xT[:, 0, :], rhs=wc1_t[:, 0, :],
                                 start=True, stop=False)
                nc.tensor.matmul(out=pn[:], lhsT=xT[:, 1, :], rhs=wc1_t[:, 1, :],
                                 start=False, stop=not use_bc1)
                if use_bc1:
                    nc.tensor.matmul(out=pn[:], lhsT=ones[:], rhs=bc1_r,
                                     start=False, stop=True)
                jn = junkp.tile([P, HC], F32, tag="junk")
                nc.vector.scalar_tensor_tensor(
                    out=jn[:], in0=pn[:], scalar=0.0, in1=wc2_rep[:],
                    op0=OP.max, op1=OP.mult, accum_out=v_all[:, t:t + 1])

            def actor_l1():
                for j in range(4):
                    pl1 = pa.tile([P, BL], F32, tag="pa")
                    nc.tensor.matmul(out=pl1[:],
                                     lhsT=wa1_t[:, 0, j * 128:(j + 1) * 128],
                                     rhs=selT[:, 0, :], start=True, stop=False)
                    nc.tensor.matmul(out=pl1[:],
                                     lhsT=wa1_t[:, 1, j * 128:(j + 1) * 128],
                                     rhs=selT[:, 1, :], start=False, stop=True)
                    nc.scalar.activation(out=ha[:, j, :], in_=pl1[:], func=AF.Relu,
                                         bias=aux_t[:, 1 + j:2 + j])

            def actor_chunk(j):
                asl = slice(j * 512, (j + 1) * 512)
                pl2 = pm.tile([BL, 512], F32, tag="mm")
                for h in range(4):
                    nc.tensor.matmul(out=pl2[:], lhsT=ha[:, h, :],
                                     rhs=wa2_t[:, h, asl],
                                     start=(h == 0), stop=(h == 3 and not use_ba2))
                if use_ba2:
                    nc.tensor.matmul(out=pl2[:], lhsT=ones[:, :BL],
                                     rhs=ba2_r[:, asl], start=False, stop=True)
                mterm = junkp.tile([BL, 512], F32, tag="mterm")
                nc.scalar.activation(out=mterm[:], in_=masks_sb[:, asl],
                                     func=AF.Copy, scale=1e10, bias=-1e10)
                nc.vector.tensor_tensor(out=ml_all[:, asl], in0=pl2[:],
                                        in1=mterm[:], op=OP.add)
                nc.sync.dma_start(out=ml_view[:, asl], in_=ml_all[:, asl])
                ej = work.tile([BL, 512], F32, tag="ej")
                nc.scalar.activation(out=ej[:], in_=ml_all[:, asl], func=AF.Exp,
                                     bias=nm0[:BL, :1], accum_out=s_all[:, j:j + 1])
                ju = junkp.tile([BL, 512], F32, tag="mterm")
                nc.vector.scalar_tensor_tensor(
                    out=ju[:], in0=ej[:], scalar=1.0, in1=ml_all[:, asl],
                    op0=OP.mult, op1=OP.mult, accum_out=u_all[:, j:j + 1])

            actor_l1()
            junk_mm()
            actor_chunk(0)
            junk_mm()
            actor_chunk(1)
            junk_mm()
            critic_tile(0)
            for j in range(2, ACH):
                actor_chunk(j)
                junk_mm()
                critic_tile(2 * j - 3)
                critic_tile(2 * j - 2)
            for t in range(13, NT):
                critic_tile(t)

            # seg-max epilogue (rows k-major: halves hold even/odd k)
            vmax = small.tile([P, 1], F32)
            nc.vector.tensor_reduce(out=vmax[:], in_=v_all[:],
                                    axis=mybir.AxisListType.X, op=OP.max)
            vmax_b = small.tile([P, 1], BF16)
            nc.vector.tensor_copy(out=vmax_b[:], in_=vmax[:])
            vhi_p = pa.tile([BL, 1], F32, tag="pa")
            nc.tensor.matmul(out=vhi_p[:], lhsT=ident[:, BL:P],
                             rhs=vmax_b[:], start=True, stop=True)
            nv1 = small.tile([BL, 1], F32)
            nc.vector.tensor_tensor(out=nv1[:], in0=vmax[0:BL, :],
                                    in1=vhi_p[:], op=OP.max)
            tf = small.tile([BL, 1], F32)
            nc.scalar.activation(out=tf[:], in_=aux_t[:BL, 5:6], func=AF.Copy,
                                 scale=-1.0, bias=1.0)
            nc.vector.scalar_tensor_tensor(
                out=out5_t[:, 1:2], in0=nv1[:], scalar=bc2_rep[:BL, :1], in1=tf[:],
                op0=OP.add, op1=OP.mult)

            # actor final reductions (log/div happen on host)
            nc.vector.tensor_reduce(out=out5_t[:, 3:4], in_=s_all[:],
                                    axis=mybir.AxisListType.X, op=OP.add)
            nc.vector.tensor_reduce(out=out5_t[:, 4:5], in_=u_all[:],
                                    axis=mybir.AxisListType.X, op=OP.add)
            nc.gpsimd.indirect_dma_start(
                out=out5_t[:, 2:3], out_offset=None, in_=ml_dram[:, :],
                in_offset=IndirectOffsetOnAxis(ap=xf_i, axis=0))
            nc.scalar.dma_start(out=out5[:, :], in_=out5_t[:])

    nc.compile()
    return nc


def _get_compiled(use_bc1, use_ba2):
    key = (use_bc1, use_ba2)
    if key not in _COMPILED:
        _COMPILED[key] = _build(use_bc1, use_ba2)
    return _COMPILED[key]


def _to_bf16(a):
    import ml_dtypes
    return np.ascontiguousarray(np.asarray(a, np.float32).astype(ml_dtypes.bfloat16))


def _make_in_maps(graph_embeds, next_graph_embeds, Wc1, bc1, Wc2, bc2,
                  Wa1, ba1, Wa2, ba2, nodes, xfers, next_node_lists,
                  is_terminals, masks):
    geb = _to_bf16(graph_embeds)
    ngeb = _to_bf16(next_graph_embeds)
    masks_u8 = np.ascontiguousarray(masks).astype(np.uint8)
    term_f = np.ascontiguousarray(is_terminals).astype(np.float32)
    nodes = np.asarray(nodes, dtype=np.int32)
    xfers = np.asarray(xfers, dtype=np.int32)
    nnl = np.asarray(next_node_lists, dtype=np.int32)
    wc1b, wa1b, wa2b = _to_bf16(Wc1), _to_bf16(Wa1), _to_bf16(Wa2)
    rowc = np.concatenate([
        np.asarray(Wc2, np.float32).ravel(),
        np.asarray(bc2, np.float32).ravel()]).astype(np.float32)
    biasb = _to_bf16(np.concatenate([
        np.asarray(bc1, np.float32).ravel(),
        np.asarray(ba2, np.float32).ravel()]))
    ba1_pm = np.asarray(ba1, np.float32).reshape(4, P).T  # [128, 4]
    eye_bf = _to_bf16(np.eye(P, dtype=np.float32))

    in_maps = []
    for c in range(NCORES):
        bs = slice(c * BL, (c + 1) * BL)
        b_loc = np.arange(BL, dtype=np.int32)
        sel = b_loc * N + nodes[bs]                       # [64]
        nextf = (b_loc[None, :] * N + nnl[bs].T).reshape(-1)  # [2048] k-major
        aux = np.zeros((P, 23), np.float32)
        auxi = aux.view(np.int32)
        auxi[:BL, 0] = b_loc * A + xfers[bs]
        aux[:, 1:5] = ba1_pm
        aux[:BL, 5] = term_f[bs]
        auxi[:BL, 6] = sel
        auxi[:, 7:23] = nextf.reshape(NT, 128).T
        in_maps.append({
            "ge": geb[c * BL * N:(c + 1) * BL * N],
            "nge": ngeb[c * BL * N:(c + 1) * BL * N],
            "wc1": wc1b, "wa1": wa1b, "wa2": wa2b,
            "masks": masks_u8[bs],
            "aux32": aux, "rowc": rowc, "biasb": biasb, "ident_d": eye_bf,
        })
    return in_maps


def kernel(**inputs):
    use_bc1 = bool(np.any(np.asarray(inputs["bc1"])))
    use_ba2 = bool(np.any(np.asarray(inputs["ba2"])))
    nc = _get_compiled(use_bc1, use_ba2)
    in_maps = _make_in_maps(**inputs)
    r = run_bass_kernel_spmd(nc, in_maps, core_ids=list(range(NCORES)))
    o = np.concatenate([r.results[c]["out5"] for c in range(NCORES)])  # [512,5]
    values, next_values = o[:, 0], o[:, 1]
    xl = o[:, 2].astype(np.float64)
    S = o[:, 3].astype(np.float64)
    U = o[:, 4].astype(np.float64)
    lse = M0 + np.log(S)
    xlp = (xl - lse).astype(np.float32)
    ent_all = lse - U / S
    xfer_entropy = np.float32(ent_all.mean())
    return (values.astype(np.float32), next_values.astype(np.float32),
            xlp, xfer_entropy)


# revision 44
# speedup vs baseline: 1.0781x; 1.0781x over previous
"""ActorCritic segment-reduce kernel for 8 TRN2 NeuronCores.

Strategy (data-parallel over graph batch B=512 -> 64 graphs/core):
  - The critic MLP is evaluated ONLY on gathered rows (64 selected-node
    rows + 2048 next-node rows per core), not on all 102400 node rows -
    that sparsity is the headroom of this problem.
  - Embeddings and weights are cast to bf16 (matmul compute precision with
    f32 PSUM accumulation; ~3e-3 rel err vs the 2e-2 gate). The segment
    gathers are per-128-row indirect DMAs on the gpsimd SWDGE; gathered
    rows are PE-transposed so the contract dim lands on partitions.
  - Critic layer 2 (relu(H) @ Wc2) is one fused DVE scalar_tensor_tensor
    per row tile: out=(max(psum,0))*Wc2_rep with accum_out emitting the
    row dot product straight from PSUM - no separate relu/mul/reduce.
  - Segment max over K=32 next nodes: the host orders gather rows k-major
    (row = k*64 + b) so the seg-max becomes one free-dim reduce over 16
    stacked result columns plus a single cross-partition-half max.
  - Actor softmax uses a fixed shift exp(ml - 16) so the exponentials fuse
    into the same pass as each 512-column logit chunk; the scalar-engine
    Exp op emits the row sum via accum_out. logp[b, xfers[b]] is a 4-byte
    indirect gather from a DRAM round-trip of the masked logits. The final
    log()/divide on [B]-sized reductions happens on the host during
    unsharding (together with the entropy mean over B).
  - Actor column chunks are interleaved between critic row tiles so the PE
    instruction stream stays dense; ~24 junk matmuls at kernel start plus
    small junk bursts in the early ramp keep the PE HAM clock at 2.4 GHz.
  - All loads are consolidated into one DMA per tensor, split across the
    sync and scalar HWDGE rings, ordered so the gather indices and
    identity land first; rank-1 (ones x bias) bias matmuls are compiled
    only when the bias vector is nonzero (they are zero in practice).
  - No cross-core communication at all: per-core [64, 5] outputs
    (values, next_values, raw xfer logit, sum-exp, sum e*ml) are
    concatenated and combined on the host.
"""
import numpy as np

import concourse.bass as bass
import concourse.mybir as mybir
import concourse.tile as tile
from concourse import bacc
from concourse.bass import IndirectOffsetOnAxis
from concourse.bass_utils import run_bass_kernel_spmd

F32 = mybir.dt.float32
BF16 = mybir.dt.bfloat16
I32 = mybir.dt.int32
U8 = mybir.dt.uint8
AF = mybir.ActivationFunctionType
OP = mybir.AluOpType

B, N, D = 512, 200, 256
HC, HA, A = 512, 512, 4096
K = 32
NCORES = 8
BL = B // NCORES            # 64 graphs per core
RN = BL * K                 # 2048 gathered next rows per core
NT = RN // 128              # 16 row tiles of gathered next rows
P = 128
ACH = A // 512              # 8 actor column chunks of 512
M0 = 16.0                   # fixed log-sum-exp shift (logits are O(5))

_COMPILED = {}


def _build(use_bc1, use_ba2):
    nc = bacc.Bacc("TRN2", target_bir_lowering=False, debug=False,
                   num_devices=NCORES)

    ge = nc.dram_tensor("ge", [BL * N, D], BF16, kind="ExternalInput")
    nge = nc.dram_tensor("nge", [BL * N, D], BF16, kind="ExternalInput")
    wc1 = nc.dram_tensor("wc1", [D, HC], BF16, kind="ExternalInput")
    wa1 = nc.dram_tensor("wa1", [D, HA], BF16, kind="ExternalInput")
    wa2 = nc.dram_tensor("wa2", [HA, A], BF16, kind="ExternalInput")
    masks = nc.dram_tensor("masks", [BL, A], U8, kind="ExternalInput")
    # aux32: col0 xf_idx (i32), cols1-4 ba1 partition-major, col5 term,
    # col6 sel_idx (i32), cols7-22 next-tile idx (i32)
    aux32 = nc.dram_tensor("aux32", [P, 23], F32, kind="ExternalInput")
    # rowc: wc2(512) | bc2(1)
    rowc = nc.dram_tensor("rowc", [513], F32, kind="ExternalInput")
    # biasb: bc1(512) | ba2(4096) in bf16
    biasb = nc.dram_tensor("biasb", [4608], BF16, kind="ExternalInput")
    ident_d = nc.dram_tensor("ident_d", [P, P], BF16, kind="ExternalInput")

    out5 = nc.dram_tensor("out5", [BL, 5], F32, kind="ExternalOutput")

    ml_dram = nc.dram_tensor("ml_dram", [BL * A, 1], F32)  # internal

    with tile.TileContext(nc) as tc:
        with (
            tc.tile_pool(name="const", bufs=1) as const,
            tc.tile_pool(name="work", bufs=6) as work,
            tc.tile_pool(name="junk", bufs=4) as junkp,
            tc.tile_pool(name="small", bufs=8) as small,
            tc.tile_pool(name="pm", bufs=4, space="PSUM") as pm,
            tc.tile_pool(name="pt", bufs=2, space="PSUM") as pt,
            tc.tile_pool(name="pa", bufs=2, space="PSUM") as pa,
        ):
            # ---- PE warm-up + ACT table preload ----
            wdum = const.tile([P, 512], BF16)
            nc.vector.memset(wdum[:], 0.0)
            pdum = pm.tile([P, 512], F32, tag="mm")
            for _ in range(24):
                nc.tensor.matmul(out=pdum[:], lhsT=wdum[:, 0:P], rhs=wdum[:],
                                 start=True, stop=True)
            dl = small.tile([1, 2], F32)
            nc.vector.memset(dl[:], 2.0)
            nc.scalar.activation(out=dl[:, 0:1], in_=dl[:, 1:2], func=AF.Exp)

            # ---- consolidated loads ----
            aux_t = const.tile([P, 23], F32)
            nc.sync.dma_start(out=aux_t[:], in_=aux32[:, :])
            ident = const.tile([P, P], BF16)
            nc.sync.dma_start(out=ident[:], in_=ident_d[:, :])
            wc2_rep = const.tile([P, HC], F32)
            nc.sync.dma_start(out=wc2_rep[:], in_=bass.AP(
                tensor=rowc, offset=0, ap=[[0, P], [1, HC]]))
            bc2_rep = const.tile([P, 1], F32)
            nc.sync.dma_start(out=bc2_rep[:], in_=bass.AP(
                tensor=rowc, offset=HC, ap=[[0, P], [1, 1]]))
            wc1_t = const.tile([P, 2, HC], BF16)
            nc.sync.dma_start(out=wc1_t[:],
                              in_=wc1.ap().rearrange("(c p) h -> p c h", p=P))
            masks_sb = const.tile([BL, A], U8)
            nc.sync.dma_start(out=masks_sb[:], in_=masks[:, :])
            bias_t = const.tile([1, 4608], BF16)
            nc.sync.dma_start(out=bias_t[:], in_=biasb[None, :])
            wa2_t = const.tile([P, 4, A], BF16)
            wa2_r = wa2.ap().rearrange("(c p) a -> p c a", p=P)
            nc.sync.dma_start(out=wa2_t[:, :, 2048:3072], in_=wa2_r[:, :, 2048:3072])
            nc.sync.dma_start(out=wa2_t[:, :, 3072:4096], in_=wa2_r[:, :, 3072:4096])
            wa1_t = const.tile([P, 2, HA], BF16)
            nc.scalar.dma_start(out=wa1_t[:],
                                in_=wa1.ap().rearrange("(c p) h -> p c h", p=P))
            nc.scalar.dma_start(out=wa2_t[:, :, 0:1024], in_=wa2_r[:, :, 0:1024])
            nc.scalar.dma_start(out=wa2_t[:, :, 1024:2048], in_=wa2_r[:, :, 1024:2048])

            bc1_r = bias_t[:, 0:HC]
            ba2_r = bias_t[:, HC:HC + A]
            xf_i = aux_t[:BL, 0:1].bitcast(I32)
            sel_i = aux_t[:BL, 6:7].bitcast(I32)
            nidx_i = [aux_t[:, 7 + t:8 + t].bitcast(I32) for t in range(NT)]
            ones = const.tile([1, P], BF16)
            nc.vector.memset(ones[:], 1.0)
            nm0 = const.tile([P, 1], F32)
            nc.vector.memset(nm0[:], -M0)


            # ---- gathers: plain indirect DMA (no ucode library), one call
            # per 128 rows; bf16 rows then PE-transposed per tile ----
            xsel = const.tile([BL, D], BF16)
            nc.gpsimd.indirect_dma_start(
                out=xsel[:], out_offset=None, in_=ge[:, :],
                in_offset=IndirectOffsetOnAxis(ap=sel_i, axis=0))
            xrows = []
            for t6 in range(NT):
                xrt = const.tile([P, D], BF16, tag=f"xr{t6}")
                xrows.append(xrt)
                nc.gpsimd.indirect_dma_start(
                    out=xrt[:], out_offset=None, in_=nge[:, :],
                    in_offset=IndirectOffsetOnAxis(ap=nidx_i[t6], axis=0))

            def junk_mm(n=2):
                pj = pm.tile([P, 512], F32, tag="mm")
                for _ in range(n):
                    nc.tensor.matmul(out=pj[:], lhsT=wdum[:, 0:P], rhs=wdum[:],
                                     start=True, stop=True)

            selT_t = const.tile([P, 2, BL], BF16)
            tsel = pt.tile([P, 2, BL], BF16, tag="tp")
            for c in range(2):
                nc.tensor.transpose(out=tsel[:, c, :],
                                    in_=xsel[:, c * P:(c + 1) * P],
                                    identity=ident[:BL, :BL])
            nc.vector.tensor_copy(out=selT_t[:], in_=tsel[:])
            selT = selT_t[:, :, :]

            # =============== critic on sel rows (values) ========
            out5_t = small.tile([BL, 5], F32)
            ps = pm.tile([BL, HC], F32, tag="mm")
            nc.tensor.matmul(out=ps[:], lhsT=selT[:, 0, :], rhs=wc1_t[:, 0, :],
                             start=True, stop=False)
            nc.tensor.matmul(out=ps[:], lhsT=selT[:, 1, :], rhs=wc1_t[:, 1, :],
                             start=False, stop=not use_bc1)
            if use_bc1:
                nc.tensor.matmul(out=ps[:], lhsT=ones[:, :BL], rhs=bc1_r,
                                 start=False, stop=True)
            jt = junkp.tile([P, HC], F32, tag="junk")
            vsel = small.tile([BL, 1], F32)
            nc.vector.scalar_tensor_tensor(
                out=jt[:BL, :], in0=ps[:], scalar=0.0, in1=wc2_rep[:BL, :],
                op0=OP.max, op1=OP.mult, accum_out=vsel[:])
            nc.vector.tensor_scalar_add(out5_t[:, 0:1], vsel[:], bc2_rep[:BL, :1])

            # ====== critic next tiles interleaved with actor chunks ======
            v_all = small.tile([P, NT], F32)
            ml_all = const.tile([BL, A], F32)
            s_all = small.tile([BL, ACH], F32)
            u_all = small.tile([BL, ACH], F32)
            ml_view = ml_dram.ap().rearrange("(b a) one -> b (a one)", b=BL)
            ha = const.tile([P, 4, BL], BF16)  # H_a laid out [h, b]

            def critic_tile(t):
                xw = work.tile([P, 2, P], BF16, tag="xt")
                tpn = pt.tile([P, 2, P], BF16, tag="tp")
                for c in range(2):
                    nc.tensor.transpose(
                        out=tpn[:, c, :],
                        in_=xrows[t][:, c * P:(c + 1) * P],
                        identity=ident[:])
                nc.vector.tensor_copy(out=xw[:], in_=tpn[:])
                xT = xw
                pn = pm.tile([P, HC], F32, tag="mm")
                nc.tensor.matmul(out=pn[:], lhsT=xT[:, 0, :], rhs=wc1_t[:, 0, :],
                                 start=True, stop=False)
                nc.tensor.matmul(out=pn[:], lhsT=xT[:, 1, :], rhs=wc1_t[:, 1, :],
                                 start=False, stop=not use_bc1)
                if use_bc1:
                    nc.tensor.matmul(out=pn[:], lhsT=ones[:], rhs=bc1_r,
                                     start=False, stop=True)
                jn = junkp.tile([P, HC], F32, tag="junk")
                nc.vector.scalar_tensor_tensor(
                    out=jn[:], in0=pn[:], scalar=0.0, in1=wc2_rep[:],
                    op0=OP.max, op1=OP.mult, accum_out=v_all[:, t:t + 1])

            def actor_l1():
                for j in range(4):
                    pl1 = pa.tile([P, BL], F32, tag="pa")
                    nc.tensor.matmul(out=pl1[:],
                                     lhsT=wa1_t[:, 0, j * 128:(j + 1) * 128],
                                     rhs=selT[:, 0, :], start=True, stop=False)
                    nc.tensor.matmul(out=pl1[:],
                                     lhsT=wa1_t[:, 1, j * 128:(j + 1) * 128],
                                     rhs=selT[:, 1, :], start=False, stop=True)
                    nc.scalar.activation(out=ha[:, j, :], in_=pl1[:], func=AF.Relu,
                                         bias=aux_t[:, 1 + j:2 + j])

            def actor_chunk(j):
                asl = slice(j * 512, (j + 1) * 512)
                pl2 = pm.tile([BL, 512], F32, tag="mm")
                for h in range(4):
                    nc.tensor.matmul(out=pl2[:], lhsT=ha[:, h, :],
                                     rhs=wa2_t[:, h, asl],
                                     start=(h == 0), stop=(h == 3 and not use_ba2))
                if use_ba2:
                    nc.tensor.matmul(out=pl2[:], lhsT=ones[:, :BL],
                                     rhs=ba2_r[:, asl], start=False, stop=True)
                mterm = junkp.tile([BL, 512], F32, tag="mterm")
                nc.scalar.activation(out=mterm[:], in_=masks_sb[:, asl],
                                     func=AF.Copy, scale=1e10, bias=-1e10)
                nc.vector.tensor_tensor(out=ml_all[:, asl], in0=pl2[:],
                                        in1=mterm[:], op=OP.add)
                nc.sync.dma_start(out=ml_view[:, asl], in_=ml_all[:, asl])
                ej = work.tile([BL, 512], F32, tag="ej")
                nc.scalar.activation(out=ej[:], in_=ml_all[:, asl], func=AF.Exp,
                                     bias=nm0[:BL, :1], accum_out=s_all[:, j:j + 1])
                ju = junkp.tile([BL, 512], F32, tag="mterm")
                nc.vector.scalar_tensor_tensor(
                    out=ju[:], in0=ej[:], scalar=1.0, in1=ml_all[:, asl],
                    op0=OP.mult, op1=OP.mult, accum_out=u_all[:, j:j + 1])

            actor_l1()
            junk_mm()
            actor_chunk(0)
            junk_mm()
            actor_chunk(1)
            junk_mm()
            critic_tile(0)
            for j in range(2, ACH):
                actor_chunk(j)
                junk_mm()
                critic_tile(2 * j - 3)
                critic_tile(2 * j - 2)
            for t in range(13, NT):
                critic_tile(t)

            # seg-max epilogue (rows k-major: halves hold even/odd k)
            vmax = small.tile([P, 1], F32)
            nc.vector.tensor_reduce(out=vmax[:], in_=v_all[:],
                                    axis=mybir.AxisListType.X, op=OP.max)
            vmax_b = small.tile([P, 1], BF16)
            nc.vector.tensor_copy(out=vmax_b[:], in_=vmax[:])
            vhi_p = pa.tile([BL, 1], F32, tag="pa")
            nc.tensor.matmul(out=vhi_p[:], lhsT=ident[:, BL:P],
                             rhs=vmax_b[:], start=True, stop=True)
            nv1 = small.tile([BL, 1], F32)
            nc.vector.tensor_tensor(out=nv1[:], in0=vmax[0:BL, :],
                                    in1=vhi_p[:], op=OP.max)
            tf = small.tile([BL, 1], F32)
            nc.scalar.activation(out=tf[:], in_=aux_t[:BL, 5:6], func=AF.Copy,
                                 scale=-1.0, bias=1.0)
            nc.vector.scalar_tensor_tensor(
                out=out5_t[:, 1:2], in0=nv1[:], scalar=bc2_rep[:BL, :1], in1=tf[:],
                op0=OP.add, op1=OP.mult)

            # actor final reductions (log/div happen on host)
            nc.vector.tensor_reduce(out=out5_t[:, 3:4], in_=s_all[:],
                                    axis=mybir.AxisListType.X, op=OP.add)
            nc.vector.tensor_reduce(out=out5_t[:, 4:5], in_=u_all[:],
                                    axis=mybir.AxisListType.X, op=OP.add)
            nc.gpsimd.indirect_dma_start(
                out=out5_t[:, 2:3], out_offset=None, in_=ml_dram[:, :],
                in_offset=IndirectOffsetOnAxis(ap=xf_i, axis=0))
            nc.scalar.dma_start(out=out5[:, :], in_=out5_t[:])

    nc.compile()
    return nc


def _get_compiled(use_bc1, use_ba2):
    key = (use_bc1, use_ba2)
    if key not in _COMPILED:
        _COMPILED[key] = _build(use_bc1, use_ba2)
    return _COMPILED[key]


def _to_bf16(a):
    import ml_dtypes
    return np.ascontiguousarray(np.asarray(a, np.float32).astype(ml_dtypes.bfloat16))


def _make_in_maps(graph_embeds, next_graph_embeds, Wc1, bc1, Wc2, bc2,
                  Wa1, ba1, Wa2, ba2, nodes, xfers, next_node_lists,
                  is_terminals, masks):
    geb = _to_bf16(graph_embeds)
    ngeb = _to_bf16(next_graph_embeds)
    masks_u8 = np.ascontiguousarray(masks).astype(np.uint8)
    term_f = np.ascontiguousarray(is_terminals).astype(np.float32)
    nodes = np.asarray(nodes, dtype=np.int32)
    xfers = np.asarray(xfers, dtype=np.int32)
    nnl = np.asarray(next_node_lists, dtype=np.int32)
    wc1b, wa1b, wa2b = _to_bf16(Wc1), _to_bf16(Wa1), _to_bf16(Wa2)
    rowc = np.concatenate([
        np.asarray(Wc2, np.float32).ravel(),
        np.asarray(bc2, np.float32).ravel()]).astype(np.float32)
    biasb = _to_bf16(np.concatenate([
        np.asarray(bc1, np.float32).ravel(),
        np.asarray(ba2, np.float32).ravel()]))
    ba1_pm = np.asarray(ba1, np.float32).reshape(4, P).T  # [128, 4]
    eye_bf = _to_bf16(np.eye(P, dtype=np.float32))

    in_maps = []
    for c in range(NCORES):
        bs = slice(c * BL, (c + 1) * BL)
        b_loc = np.arange(BL, dtype=np.int32)
        sel = b_loc * N + nodes[bs]                       # [64]
        nextf = (b_loc[None, :] * N + nnl[bs].T).reshape(-1)  # [2048] k-major
        aux = np.zeros((P, 23), np.float32)
        auxi = aux.view(np.int32)
        auxi[:BL, 0] = b_loc * A + xfers[bs]
        aux[:, 1:5] = ba1_pm
        aux[:BL, 5] = term_f[bs]
        auxi[:BL, 6] = sel
        auxi[:, 7:23] = nextf.reshape(NT, 128).T
        in_maps.append({
            "ge": geb[c * BL * N:(c + 1) * BL * N],
            "nge": ngeb[c * BL * N:(c + 1) * BL * N],
            "wc1": wc1b, "wa1": wa1b, "wa2": wa2b,
            "masks": masks_u8[bs],
            "aux32": aux, "rowc": rowc, "biasb": biasb, "ident_d": eye_bf,
        })
    return in_maps


def kernel(**inputs):
    use_bc1 = bool(np.any(np.asarray(inputs["bc1"])))
    use_ba2 = bool(np.any(np.asarray(inputs["ba2"])))
    nc = _get_compiled(use_bc1, use_ba2)
    in_maps = _make_in_maps(**inputs)
    r = run_bass_kernel_spmd(nc, in_maps, core_ids=list(range(NCORES)))
    o = np.concatenate([r.results[c]["out5"] for c in range(NCORES)])  # [512,5]
    values, next_values = o[:, 0], o[:, 1]
    xl = o[:, 2].astype(np.float64)
    S = o[:, 3].astype(np.float64)
    U = o[:, 4].astype(np.float64)
    lse = M0 + np.log(S)
    xlp = (xl - lse).astype(np.float32)
    ent_all = lse - U / S
    xfer_entropy = np.float32(ent_all.mean())
    return (values.astype(np.float32), next_values.astype(np.float32),
            xlp, xfer_entropy)
